# Initial kernel scaffold
#
"""Multi-class DICE loss on 8 Trainium2 NeuronCores.

Reference computation (B=16, C=8, H=W=512):
    onehot = (mask[:,None] == arange(C))        # [B,C,H,W]
    num  = sum(output * onehot, axis=(2,3))     # [B,C]
    den1 = sum(output * output, axis=(2,3))     # [B,C]
    den2 = sum(onehot, axis=(2,3))              # [B,C]
    dice = 2 * (num + eps) / (den1 + den2 + eps)
    loss = 1 - sum(dice) / (B*B)

Sharding: pure data parallel over batch; each of 8 cores takes 2
samples (16 (b,c) class-tiles of [128, 2048]). Per class-tile:
  DVE  scalar_tensor_tensor (mask==c)*x with fused accum -> num partial
  DVE  tensor_scalar (mask==c) in bf16 (4x mode) w/ accum -> den2 partial
  ACT  activation Square with fused accum                 -> den1 partial
The int32 mask is converted to bf16 labels once per sample on Pool.
Per-partition partials [128, 16] are folded to [1, 16] with one
ones-vector PE matmul each; dice is evaluated on partition 0 and the
core's dice-sum is written out. The 8 per-core partial sums are added
on the host (the unshard step) - no device collective.
"""

import os
from contextlib import ExitStack

import numpy as np

import concourse.bacc as bacc
import concourse.bass as bass
import concourse.tile as tile
from concourse import mybir
from concourse.bass_utils import run_bass_kernel_spmd

N_CORES = 8
B, C, H, W = 16, 8, 512, 512
B_LOC = B // N_CORES          # samples per core
HWPIX = H * W                 # 262144 pixels per (b, c)
P = 128                       # SBUF partitions
NCOL = HWPIX // P             # 2048 free-dim columns per class-tile
ROWS = B_LOC * C              # 16 (b, c) pairs per core
G = 4                         # classes per x DMA group
EPS = 1e-7


_cache: dict = {}
last_results = None           # BassKernelResults of the most recent run


def _build(mask64: bool, collective: bool = False) -> bass.Bass:
    nc = bacc.Bacc(
        "TRN2",
        target_bir_lowering=False,
        debug=False,
        num_devices=N_CORES if collective else 1,
    )
    f32 = mybir.dt.float32
    bf16 = mybir.dt.bfloat16
    i32 = mybir.dt.int32

    x = nc.dram_tensor("x", [ROWS, P, NCOL], f32, kind="ExternalInput")
    # int64 masks arrive as little-endian int32 pairs; low word holds the
    # label, extracted with a stride-2 access pattern on chip.
    m_cols = NCOL * 2 if mask64 else NCOL
    m = nc.dram_tensor("m", [B_LOC, P, m_cols], i32, kind="ExternalInput")
    part = nc.dram_tensor("part", [1], f32, kind="ExternalOutput")

    with tile.TileContext(nc) as tc, ExitStack() as ctx:
        xpool = ctx.enter_context(tc.tile_pool(name="xp", bufs=4))
        mpool = ctx.enter_context(tc.tile_pool(name="mp", bufs=2))
        mfpool = ctx.enter_context(tc.tile_pool(name="mfp", bufs=2))
        jpool = ctx.enter_context(tc.tile_pool(name="jp", bufs=2))
        epool = ctx.enter_context(tc.tile_pool(name="ep", bufs=4))
        spool = ctx.enter_context(tc.tile_pool(name="sp", bufs=1))
        acc = ctx.enter_context(tc.tile_pool(name="acc", bufs=1))
        pspool = ctx.enter_context(tc.tile_pool(name="ps", bufs=1, space="PSUM"))

        ones32 = acc.tile([P, 1], f32, tag="ones32")
        nc.vector.memset(ones32, 1.0)
        onesb = acc.tile([P, 1], bf16, tag="onesb")
        nc.vector.memset(onesb, 1.0)

        # Per-partition partial sums, one column per (b, c) pair.
        p_num = acc.tile([P, ROWS], f32, tag="p_num")
        p_den1 = acc.tile([P, ROWS], f32, tag="p_den1")
        # den2 accumulates as finished scalars on partition 0.
        den2row = acc.tile([1, ROWS], f32, tag="den2row")

        for b in range(B_LOC):
            mraw = mpool.tile([P, m_cols], i32, tag="mraw")
            nc.sync.dma_start(out=mraw, in_=m[b])
            if mask64:
                msrc = mraw.rearrange("p (n two) -> p n two", two=2)[:, :, 0]
            else:
                msrc = mraw[:]
            # int32 -> bf16 label copy; labels 0..7 are exact. Runs on
            # DVE (1.2us): the Pool engine's software cast (7us) causes
            # heavy SBUF port contention that stretches whichever DVE op
            # overlaps it by ~4x.
            mf = mfpool.tile([P, NCOL], bf16, tag="mf")
            nc.vector.tensor_copy(out=mf, in_=msrc)

            # The first sample's x tiles arrive staircased (1, 3, 4
            # classes) so compute starts as soon as 1 MB has landed;
            # steady state uses 4-class (4 MB) groups.
            groups = [1, 3, G] if b == 0 else [G, G]
            c0 = 0
            for gsz in groups:
                xt = xpool.tile([P, G, NCOL], f32, tag="xt")
                nc.sync.dma_start(
                    out=xt[:, 0:gsz, :],
                    in_=x[b * C + c0 : b * C + c0 + gsz].transpose([1, 0, 2]),
                )
                for i in range(gsz):
                    c = c0 + i
                    col = b * C + c
                    # eq = (mask == c) in bf16 (DVE 4x mode)
                    eq = epool.tile([P, NCOL], bf16, tag="eq")
                    nc.vector.tensor_scalar(
                        out=eq,
                        in0=mf,
                        scalar1=float(c),
                        scalar2=None,
                        op0=mybir.AluOpType.is_equal,
                    )
                    # num partial: (mask == c) * x, accumulated per partition
                    junk = jpool.tile([P, NCOL], mybir.dt.float8e4, tag="junk")
                    nc.vector.scalar_tensor_tensor(
                        out=junk,
                        in0=mf,
                        scalar=float(c),
                        in1=xt[:, i, :],
                        op0=mybir.AluOpType.is_equal,
                        op1=mybir.AluOpType.mult,
                        accum_out=p_num[:, col : col + 1],
                    )
                    # den1 partial: x^2, accumulated per partition.
                    # Emitted before the den2 mini-reduce: the in-order
                    # ACT queue would otherwise stall the Square behind
                    # a mini-reduce still waiting on the PE chunk sums.
                    sjunk = spool.tile([P, NCOL], mybir.dt.float8e4, tag="sjunk")
                    nc.scalar.activation(
                        out=sjunk,
                        in_=xt[:, i, :],
                        func=mybir.ActivationFunctionType.Square,
                        accum_out=p_den1[:, col : col + 1],
                    )
                    # den2 on PE: ones^T @ eq chunk-accumulated into a
                    # psum row, then one small ACT Copy+accum collapses
                    # it to the den2 scalar on partition 0.
                    psd = pspool.tile([1, 512], f32, tag="psd", bufs=4)
                    nchunk = NCOL // 512
                    for j in range(nchunk):
                        nc.tensor.matmul(
                            out=psd,
                            lhsT=onesb,
                            rhs=eq[:, j * 512 : (j + 1) * 512],
                            start=(j == 0),
                            stop=(j == nchunk - 1),
                        )
                    mjunk = spool.tile([1, 512], f32, tag="mjunk")
                    nc.scalar.activation(
                        out=mjunk,
                        in_=psd[:],
                        func=mybir.ActivationFunctionType.Copy,
                        accum_out=den2row[:, col : col + 1],
                    )
                c0 += gsz

        # Fold partition dim: [128, 16] -> psum [1, 16] via ones matmul.
        ps_n = pspool.tile([1, ROWS], f32, tag="ps_n")
        nc.tensor.matmul(out=ps_n, lhsT=ones32, rhs=p_num[:], start=True, stop=True)
        ps_d1 = pspool.tile([1, ROWS], f32, tag="ps_d1")
        nc.tensor.matmul(out=ps_d1, lhsT=ones32, rhs=p_den1[:], start=True, stop=True)
        # dice = (num + eps) / (den1 + den2 + eps) on partition 0;
        # the factor 2 and the 1 - .../B^2 affine are applied on host.
        dene = acc.tile([1, ROWS], f32, tag="dene")
        nc.vector.scalar_tensor_tensor(
            out=dene,
            in0=ps_d1[:],
            scalar=EPS,
            in1=den2row,
            op0=mybir.AluOpType.add,
            op1=mybir.AluOpType.add,
        )
        rec = acc.tile([1, ROWS], f32, tag="rec")
        nc.vector.reciprocal(out=rec, in_=dene)
        nume = acc.tile([1, ROWS], f32, tag="nume")
        nc.vector.tensor_scalar_add(out=nume, in0=ps_n[:], scalar1=EPS)
        dice = acc.tile([1, ROWS], f32, tag="dice")
        nc.vector.tensor_tensor(
            out=dice, in0=nume, in1=rec, op=mybir.AluOpType.mult
        )
        lsum = acc.tile([1, 1], f32, tag="lsum")
        nc.vector.tensor_reduce(
            out=lsum, in_=dice, axis=mybir.AxisListType.X, op=mybir.AluOpType.add
        )
        nc.sync.dma_start(out=part[:], in_=lsum)

    nc.compile()
    return nc


def _get(mask64: bool) -> bass.Bass:
    if mask64 not in _cache:
        _cache[mask64] = _build(mask64)
    return _cache[mask64]


def make_in_maps(output: np.ndarray, mask: np.ndarray, mask64: bool):
    in_maps = []
    for i in range(N_CORES):
        xs = output[i * B_LOC : (i + 1) * B_LOC].reshape(ROWS, P, NCOL)
        ms = np.ascontiguousarray(mask[i * B_LOC : (i + 1) * B_LOC])
        if mask64:
            ms = ms.view(np.int32).reshape(B_LOC, P, NCOL * 2)
        else:
            ms = ms.reshape(B_LOC, P, NCOL)
        in_maps.append({"x": np.ascontiguousarray(xs), "m": ms})
    return in_maps


def kernel(output: np.ndarray, mask: np.ndarray) -> np.ndarray:
    global last_results
    output = np.ascontiguousarray(np.asarray(output, dtype=np.float32))
    mask = np.asarray(mask)
    assert output.shape == (B, C, H, W), output.shape
    assert mask.shape == (B, H, W), mask.shape
    mask64 = mask.dtype.itemsize == 8
    if not mask64 and mask.dtype != np.int32:
        mask = mask.astype(np.int32)

    nc = _get(mask64)
    in_maps = make_in_maps(output, mask, mask64)
    last_results = run_bass_kernel_spmd(
        nc,
        in_maps,
        list(range(N_CORES)),
        trace=bool(os.environ.get("DICE_TRACE")),
    )
    total = 0.0
    for r in last_results.results:
        total += float(np.asarray(r["part"]).reshape(()))
    loss = 1.0 - 2.0 * total / (B * B)
    return np.float32(loss).reshape(())



# revision 1
# speedup vs baseline: 1.2466x; 1.2466x over previous
"""Multi-class DICE loss on 8 Trainium2 NeuronCores.

Reference computation (B=16, C=8, H=W=512):
    onehot = (mask[:,None] == arange(C))        # [B,C,H,W]
    num  = sum(output * onehot, axis=(2,3))     # [B,C]
    den1 = sum(output * output, axis=(2,3))     # [B,C]
    den2 = sum(onehot, axis=(2,3))              # [B,C]
    dice = 2 * (num + eps) / (den1 + den2 + eps)
    loss = 1 - sum(dice) / (B*B)

Sharding: pure data parallel over batch; each of 8 cores takes 2
samples (16 (b,c) class-tiles of [128, 2048]). Per class-tile:
  DVE  scalar_tensor_tensor (mask==c)*x with fused accum -> num partial
  DVE  tensor_scalar (mask==c) in bf16 (4x mode) w/ accum -> den2 partial
  ACT  activation Square with fused accum                 -> den1 partial
The int32 mask is converted to bf16 labels once per sample on Pool.
Per-partition partials [128, 16] are folded to [1, 16] with one
ones-vector PE matmul each; dice is evaluated on partition 0 and the
core's dice-sum is written out. The 8 per-core partial sums are added
on the host (the unshard step) - no device collective.
"""

import os
from contextlib import ExitStack

import numpy as np

import concourse.bacc as bacc
import concourse.bass as bass
import concourse.tile as tile
from concourse import mybir
from concourse.bass_utils import run_bass_kernel_spmd

N_CORES = 8
B, C, H, W = 16, 8, 512, 512
B_LOC = B // N_CORES          # samples per core
HWPIX = H * W                 # 262144 pixels per (b, c)
P = 128                       # SBUF partitions
NCOL = HWPIX // P             # 2048 free-dim columns per class-tile
ROWS = B_LOC * C              # 16 (b, c) pairs per core
G = 4                         # classes per x DMA group
EPS = 1e-7


_cache: dict = {}
last_results = None           # BassKernelResults of the most recent run


def _build(mask64: bool, collective: bool = False) -> bass.Bass:
    nc = bacc.Bacc(
        "TRN2",
        target_bir_lowering=False,
        debug=False,
        num_devices=N_CORES if collective else 1,
    )
    f32 = mybir.dt.float32
    bf16 = mybir.dt.bfloat16
    i32 = mybir.dt.int32

    x = nc.dram_tensor("x", [ROWS, P, NCOL], f32, kind="ExternalInput")
    # int64 masks arrive as little-endian int32 pairs; low word holds the
    # label, extracted with a stride-2 access pattern on chip.
    m_cols = NCOL * 2 if mask64 else NCOL
    m = nc.dram_tensor("m", [B_LOC, P, m_cols], i32, kind="ExternalInput")
    part = nc.dram_tensor("part", [1], f32, kind="ExternalOutput")

    with tile.TileContext(nc) as tc, ExitStack() as ctx:
        xpool = ctx.enter_context(tc.tile_pool(name="xp", bufs=4))
        mpool = ctx.enter_context(tc.tile_pool(name="mp", bufs=2))
        mfpool = ctx.enter_context(tc.tile_pool(name="mfp", bufs=2))
        jpool = ctx.enter_context(tc.tile_pool(name="jp", bufs=2))
        epool = ctx.enter_context(tc.tile_pool(name="ep", bufs=4))
        spool = ctx.enter_context(tc.tile_pool(name="sp", bufs=1))
        acc = ctx.enter_context(tc.tile_pool(name="acc", bufs=1))
        pspool = ctx.enter_context(tc.tile_pool(name="ps", bufs=1, space="PSUM"))

        ones32 = acc.tile([P, 1], f32, tag="ones32")
        nc.vector.memset(ones32, 1.0)
        onesb = acc.tile([P, 1], bf16, tag="onesb")
        nc.vector.memset(onesb, 1.0)

        # Per-partition partial sums, one column per (b, c) pair.
        p_num = acc.tile([P, ROWS], f32, tag="p_num")
        p_den1 = acc.tile([P, ROWS], f32, tag="p_den1")
        # den2 accumulates as finished scalars on partition 0.
        den2row = acc.tile([1, ROWS], f32, tag="den2row")

        for b in range(B_LOC):
            mraw = mpool.tile([P, m_cols], i32, tag="mraw")
            nc.sync.dma_start(out=mraw, in_=m[b])
            if mask64:
                msrc = mraw.rearrange("p (n two) -> p n two", two=2)[:, :, 0]
            else:
                msrc = mraw[:]
            # int32 -> bf16 label copy; labels 0..7 are exact. Runs on
            # DVE (1.2us): the Pool engine's software cast (7us) causes
            # heavy SBUF port contention that stretches whichever DVE op
            # overlaps it by ~4x.
            mf = mfpool.tile([P, NCOL], bf16, tag="mf")
            nc.vector.tensor_copy(out=mf, in_=msrc)

            # The first sample's x tiles arrive staircased (1, 3, 4
            # classes) so compute starts as soon as 1 MB has landed;
            # steady state uses 4-class (4 MB) groups.
            groups = [1, 3, G] if b == 0 else [G, G]
            c0 = 0
            for gsz in groups:
                xt = xpool.tile([P, G, NCOL], f32, tag="xt")
                nc.sync.dma_start(
                    out=xt[:, 0:gsz, :],
                    in_=x[b * C + c0 : b * C + c0 + gsz].transpose([1, 0, 2]),
                )
                for i in range(gsz):
                    c = c0 + i
                    col = b * C + c
                    # eq = (mask == c) in bf16 (DVE 4x mode)
                    eq = epool.tile([P, NCOL], bf16, tag="eq")
                    nc.vector.tensor_scalar(
                        out=eq,
                        in0=mf,
                        scalar1=float(c),
                        scalar2=None,
                        op0=mybir.AluOpType.is_equal,
                    )
                    # num partial: (mask == c) * x, accumulated per partition
                    junk = jpool.tile([P, NCOL], mybir.dt.float8e4, tag="junk")
                    nc.vector.scalar_tensor_tensor(
                        out=junk,
                        in0=mf,
                        scalar=float(c),
                        in1=xt[:, i, :],
                        op0=mybir.AluOpType.is_equal,
                        op1=mybir.AluOpType.mult,
                        accum_out=p_num[:, col : col + 1],
                    )
                    # den1 partial: x^2, accumulated per partition.
                    # Emitted before the den2 mini-reduce: the in-order
                    # ACT queue would otherwise stall the Square behind
                    # a mini-reduce still waiting on the PE chunk sums.
                    sjunk = spool.tile([P, NCOL], mybir.dt.float8e4, tag="sjunk")
                    nc.scalar.activation(
                        out=sjunk,
                        in_=xt[:, i, :],
                        func=mybir.ActivationFunctionType.Square,
                        accum_out=p_den1[:, col : col + 1],
                    )
                    # den2 on PE: ones^T @ eq chunk-accumulated into a
                    # psum row, then one small ACT Copy+accum collapses
                    # it to the den2 scalar on partition 0.
                    psd = pspool.tile([1, 512], f32, tag="psd", bufs=4)
                    nchunk = NCOL // 512
                    for j in range(nchunk):
                        nc.tensor.matmul(
                            out=psd,
                            lhsT=onesb,
                            rhs=eq[:, j * 512 : (j + 1) * 512],
                            start=(j == 0),
                            stop=(j == nchunk - 1),
                        )
                    mjunk = spool.tile([1, 512], f32, tag="mjunk")
                    nc.scalar.activation(
                        out=mjunk,
                        in_=psd[:],
                        func=mybir.ActivationFunctionType.Copy,
                        accum_out=den2row[:, col : col + 1],
                    )
                c0 += gsz

        # Fold partition dim: [128, 16] -> psum [1, 16] via ones matmul.
        ps_n = pspool.tile([1, ROWS], f32, tag="ps_n")
        nc.tensor.matmul(out=ps_n, lhsT=ones32, rhs=p_num[:], start=True, stop=True)
        ps_d1 = pspool.tile([1, ROWS], f32, tag="ps_d1")
        nc.tensor.matmul(out=ps_d1, lhsT=ones32, rhs=p_den1[:], start=True, stop=True)
        # dice = (num + eps) / (den1 + den2 + eps) on partition 0;
        # the factor 2 and the 1 - .../B^2 affine are applied on host.
        dene = acc.tile([1, ROWS], f32, tag="dene")
        nc.vector.scalar_tensor_tensor(
            out=dene,
            in0=ps_d1[:],
            scalar=EPS,
            in1=den2row,
            op0=mybir.AluOpType.add,
            op1=mybir.AluOpType.add,
        )
        rec = acc.tile([1, ROWS], f32, tag="rec")
        nc.vector.reciprocal(out=rec, in_=dene)
        nume = acc.tile([1, ROWS], f32, tag="nume")
        nc.vector.tensor_scalar_add(out=nume, in0=ps_n[:], scalar1=EPS)
        dice = acc.tile([1, ROWS], f32, tag="dice")
        nc.vector.tensor_tensor(
            out=dice, in0=nume, in1=rec, op=mybir.AluOpType.mult
        )
        lsum = acc.tile([1, 1], f32, tag="lsum")
        nc.vector.tensor_reduce(
            out=lsum, in_=dice, axis=mybir.AxisListType.X, op=mybir.AluOpType.add
        )
        nc.sync.dma_start(out=part[:], in_=lsum)

    nc.compile()
    return nc


def _get(mask64: bool) -> bass.Bass:
    if mask64 not in _cache:
        _cache[mask64] = _build(mask64)
    return _cache[mask64]


def make_in_maps(output: np.ndarray, mask: np.ndarray, mask64: bool):
    in_maps = []
    for i in range(N_CORES):
        xs = output[i * B_LOC : (i + 1) * B_LOC].reshape(ROWS, P, NCOL)
        ms = np.ascontiguousarray(mask[i * B_LOC : (i + 1) * B_LOC])
        if mask64:
            ms = ms.view(np.int32).reshape(B_LOC, P, NCOL * 2)
        else:
            ms = ms.reshape(B_LOC, P, NCOL)
        in_maps.append({"x": np.ascontiguousarray(xs), "m": ms})
    return in_maps


def kernel(output: np.ndarray, mask: np.ndarray) -> np.ndarray:
    global last_results
    output = np.ascontiguousarray(np.asarray(output, dtype=np.float32))
    mask = np.asarray(mask)
    assert output.shape == (B, C, H, W), output.shape
    assert mask.shape == (B, H, W), mask.shape
    mask64 = mask.dtype.itemsize == 8
    if not mask64 and mask.dtype != np.int32:
        mask = mask.astype(np.int32)

    nc = _get(mask64)
    in_maps = make_in_maps(output, mask, mask64)
    last_results = run_bass_kernel_spmd(
        nc,
        in_maps,
        list(range(N_CORES)),
        trace=bool(os.environ.get("DICE_TRACE")),
    )
    total = 0.0
    for r in last_results.results:
        total += float(np.asarray(r["part"]).reshape(()))
    loss = 1.0 - 2.0 * total / (B * B)
    return np.float32(loss).reshape(())

